# revision 60
# baseline (speedup 1.0000x reference)
"""CausalBoW (causal mean pooling) Trainium2 Bass kernel.

y[b, t, :] = mean(x[b, 0:t+1, :]) = cumsum(x, axis=1) / (t+1)

Full input x: [8, 4096, 1024] f32. Sharded batch-parallel: one batch of
[4096, 1024] per NeuronCore (8 cores).

The fp32 input is re-encoded (losslessly up to ~2^-18 relative) as a pair
of bf16 tensors xh = bf16(x), xl = bf16(x - xh). This keeps HBM traffic
identical to streaming fp32 x (2 x 8 MiB vs 16 MiB per core) while letting
every matmul run at the PE's full 1 column/cycle bf16 rate (fp32 matmul is
4x slower, fp32r truncates to 11 mantissa bits). All matmul weights are
exactly 0/1 so the products are exact; accumulation is fp32 in PSUM.

Per-core algorithm: blocked scan, T on partitions in 32 tiles of 128 rows,
processed in 4 chunks of 8 tiles so compute/output of chunk g overlaps the
input stream of chunk g+1:
  per chunk g:
    per tile i in chunk: DMA xh/xl tile in;
      PSUM Sg[i%8, :] += colsum(xh_i) + colsum(xl_i)  (one-hot selector MM)
    evict Sg into s2 rows [g*16, g*16+8) as bf16-hi and [g*16+8, g*16+16)
      as bf16-lo  (s2 interleaves chunk-blocks of hi/lo tile sums)
    per tile i in chunk, per 512-half (PSUM accumulation group):
      z  = tri.T @ xh_i + tri.T @ xl_i        local inclusive cumsum
      z += carb2_i.T @ s2[0:16*(g+1)]         carry (hi+lo folded, one MM)
      y_i = z * (1/(t+1))    per-partition scale on PSUM->SBUF evict
      DMA y half out.
"""

import sys

for _p in ("/opt/trn_rl_repo",):
    if _p not in sys.path:
        sys.path.insert(0, _p)

import ml_dtypes
import numpy as np

import concourse.bass as bass
import concourse.mybir as mybir
import concourse.tile as tile
from concourse import bacc
from concourse.bass_utils import run_bass_kernel_spmd

B, T, C = 8, 4096, 1024
P = 128            # partition tile rows
NT = T // P        # 32 row-tiles
HALF = 512         # PSUM bank free-dim for f32
NH = C // HALF     # 2 halves
CHS = [8, 8, 8, 8]             # chunk sizes (tiles)
assert sum(CHS) == NT
COFF = [sum(CHS[:b]) for b in range(len(CHS))]   # chunk tile offsets

F32 = mybir.dt.float32
BF16 = mybir.dt.bfloat16


def _build_nc() -> bass.Bass:
    nc = bacc.Bacc(trn_type="TRN2")

    xh = nc.declare_dram_parameter("xh", [T, C], BF16, isOutput=False)
    xl = nc.declare_dram_parameter("xl", [T, C], BF16, isOutput=False)
    y = nc.declare_dram_parameter("y", [T, C], F32, isOutput=True)

    # Constants baked into the NEFF (all weights exactly 0/1).
    # lhsT for local inclusive cumsum: out = lhsT.T @ rhs, want
    # out[t, c] = sum_{s<=t} x[s, c] => lhsT[s, t] = 1 iff s <= t.
    tri_np = np.triu(np.ones((P, P), dtype=ml_dtypes.bfloat16))
    # carry weights over the interleaved tile-sum table s2:
    # s2 row k holds: chunk c0 = k//16, j8 = k%16; tile j = c0*8 + (j8%8);
    # j8 < 8 -> hi part of S_j, else lo part. carry_i needs sum of both
    # parts for all j < i:
    #   carb2[k, i*128 + m] = 1 iff (k//16)*8 + (k%16)%8 < i.
    jmap = np.empty(NT * 2, dtype=np.int64)
    for b, (off, cb) in enumerate(zip(COFF, CHS)):
        k0 = 2 * off
        jmap[k0 : k0 + 2 * cb] = off + (np.arange(2 * cb) % cb)
    carb2_np = (
        (jmap[:, None, None] < np.arange(NT)[None, :, None])
        * np.ones((1, 1, P))
    ).reshape(NT * 2, NT * P).astype(ml_dtypes.bfloat16)
    # banded one-hot-column selector for routing colsum(x_i) into PSUM row
    # j: bnd8[:, (7-j) : (7-j+cb)] has ones exactly in column j.
    bnd8_np = np.zeros((P, 15), dtype=ml_dtypes.bfloat16)
    bnd8_np[:, 7] = 1.0
    # inv[p, i] = 1 / (i*128 + p + 1)
    inv_np = (
        1.0 / np.arange(1, T + 1, dtype=np.float64)
    ).astype(np.float32).reshape(NT, P).T.copy()

    tri_d = nc.inline_tensor(tri_np, name="tri_c")
    carb2_d = nc.inline_tensor(carb2_np, name="carb2_c")
    bnd8_d = nc.inline_tensor(bnd8_np, name="bnd8_c")
    inv_d = nc.inline_tensor(inv_np, name="inv_c")

    with tile.TileContext(nc) as tc:
        with (
            tc.tile_pool(name="consts", bufs=1) as cpool,
            tc.tile_pool(name="xpool", bufs=2 * NT) as xpool,
            tc.tile_pool(name="ypoolA", bufs=4) as ypoolA,
            tc.tile_pool(name="ypoolB", bufs=4) as ypoolB,
            tc.tile_pool(name="s2p", bufs=1) as s2p,
            tc.tile_pool(name="stmp", bufs=2) as stmp,
            tc.tile_pool(name="ps_s", bufs=4, space="PSUM") as ps_s,
            tc.tile_pool(name="ps_z", bufs=4, space="PSUM") as ps_z,
        ):
            bnd8_sb = cpool.tile([P, 15], BF16)
            nc.sync.dma_start(bnd8_sb[:], bnd8_d.ap())
            tri_sb = cpool.tile([P, P], BF16)
            nc.sync.dma_start(tri_sb[:], tri_d.ap())
            inv_sb = cpool.tile([P, NT], F32)
            nc.sync.dma_start(inv_sb[:], inv_d.ap())
            carb2_sb = cpool.tile([NT * 2, NT * P], BF16)

            s2_sb = s2p.tile([NT * 2, C], BF16)

            xhs, xls = [None] * NT, [None] * NT

            def load_and_colsum(g: int, j: int, s_ps):
                """DMA tile j of chunk g in, accumulate its column sums."""
                i = COFF[g] + j
                cb = CHS[g]
                # Each DMA engine runs at ~1/16 of HBM bandwidth, so a whole
                # 256 KiB tile on one queue has ~11 us latency. Split the
                # first tiles so the pipeline starts promptly.
                nsplit = 4 if i < 1 else 1
                ps = P // nsplit
                xht = xpool.tile([P, C], BF16, name=f"xht{i}", tag="x")
                xlt = xpool.tile([P, C], BF16, name=f"xlt{i}", tag="x")
                for s in range(nsplit):
                    rs = slice(s * ps, (s + 1) * ps)
                    gs = slice(i * P + s * ps, i * P + (s + 1) * ps)
                    # first tile: fan triggers over both HWDGE engines to
                    # dodge the sync-queue trigger serialization at start
                    heng = nc.scalar if nsplit > 1 else nc.sync
                    heng.dma_start(xht[rs, :], xh.ap()[gs, :])
                    nc.sync.dma_start(xlt[rs, :], xl.ap()[gs, :])
                xhs[i], xls[i] = xht, xlt
                lhs_j = bnd8_sb[:, 7 - j : 7 - j + cb]
                for h in range(NH):
                    hs = slice(h * HALF, (h + 1) * HALF)
                    nc.tensor.matmul(
                        s_ps[h][:], lhsT=lhs_j, rhs=xht[:, hs],
                        start=(j == 0), stop=False,
                    )
                    nc.tensor.matmul(
                        s_ps[h][:], lhsT=lhs_j, rhs=xlt[:, hs],
                        start=False, stop=(j == cb - 1),
                    )

            def assemble_s2(g: int, s_ps):
                """Evict chunk-g tile-sums into s2 rows as bf16 hi/lo.

                DVE writes must start at partition 0/32/64/96, so evict to
                base-0 temporaries and DMA (any partition) into s2 rows.
                """
                cb = CHS[g]
                r0 = 2 * COFF[g]
                th = stmp.tile([cb, C], BF16, name=f"th{g}", tag="th")
                tl = stmp.tile([cb, C], BF16, name=f"tl{g}", tag="tl")
                for h in range(NH):
                    hs = slice(h * HALF, (h + 1) * HALF)
                    nc.scalar.copy(th[:, hs], s_ps[h][:])
                    nc.vector.tensor_sub(tl[:, hs], s_ps[h][:], th[:, hs])
                # scalar HWDGE: the th copy runs on ACT, so its hop trigger
                # follows in the same queue with no cross-engine sem, and
                # ACT's trigger queue is short (sync carries the input
                # stream)
                nc.scalar.dma_start(s2_sb[r0 : r0 + cb, :], th[:])
                nc.scalar.dma_start(s2_sb[r0 + cb : r0 + 2 * cb, :], tl[:])

            zps = [None] * NT

            def phase_c_tri(i: int):
                """Local-cumsum matmuls for tile i (no carry dependency)."""
                zps[i] = []
                for h in range(NH):
                    zp = ps_z.tile([P, HALF], F32, name=f"zp{i}_{h}", tag="z")
                    zps[i].append(zp)
                    hs = slice(h * HALF, (h + 1) * HALF)
                    nc.tensor.matmul(
                        zp[:], lhsT=tri_sb[:], rhs=xhs[i][:, hs],
                        start=True, stop=False,
                    )
                    nc.tensor.matmul(
                        zp[:], lhsT=tri_sb[:], rhs=xls[i][:, hs],
                        start=False, stop=(i == 0),
                    )

            def phase_c_fin(i: int):
                """Carry matmul + scale-evict + store for tile i."""
                # carry table prefix: rows for all tiles < i. The first tile
                # of a chunk needs nothing from its own chunk's block.
                b = max(bb for bb in range(len(CHS)) if COFF[bb] <= i)
                k2 = 2 * COFF[b] + (2 * CHS[b] if i > COFF[b] else 0)
                for h in range(NH):
                    zp = zps[i][h]
                    hs = slice(h * HALF, (h + 1) * HALF)
                    if i > 0:
                        nc.tensor.matmul(
                            zp[:],
                            lhsT=carb2_sb[0:k2, i * P : (i + 1) * P],
                            rhs=s2_sb[0:k2, hs],
                            start=False, stop=True,
                        )
                    # evict with per-partition 1/(t+1) scale; split halves
                    # across ACT and DVE. The h0 store is triggered from the
                    # scalar engine itself (HWDGE) to offload the sync queue.
                    if h == 0:
                        yt = ypoolA.tile([P, HALF], F32, name=f"yta{i}",
                                         tag="ya")
                        nc.scalar.mul(yt[:], zp[:], inv_sb[:, i : i + 1])
                        dma_eng = nc.gpsimd
                    else:
                        yt = ypoolB.tile([P, HALF], F32, name=f"ytb{i}",
                                         tag="yb")
                        nc.vector.tensor_scalar_mul(
                            yt[:], zp[:], inv_sb[:, i : i + 1]
                        )
                        dma_eng = nc.gpsimd
                    # tail: the input stream is done, sync is idle — use it
                    # for the last stores so they don't queue on scalar
                    if i >= NT - 2:
                        dma_eng = nc.sync
                    # split the last tiles' stores to shorten the tail
                    nsplit = 2 if i >= NT - 2 else 1
                    ps = P // nsplit
                    for s in range(nsplit):
                        rs = slice(s * ps, (s + 1) * ps)
                        gs = slice(i * P + s * ps, i * P + (s + 1) * ps)
                        dma_eng.dma_start(y.ap()[gs, hs], yt[rs, :])

            # Software pipeline: interleave chunk g's input stream + column
            # sums with chunk g-1's compute at tile granularity, so the
            # in-order PE queue always has dense work between DMA-paced
            # column-sum matmuls and the s2 assembly latency is hidden.
            # phase-C is emitted with a two-tile lag between the carry-free
            # tri matmuls and the carry+evict part, so the PE queue always
            # has independent work while the s2 carry table assembles.
            LAG = 1
            pending: list = []

            def emit_tri(i: int):
                phase_c_tri(i)
                pending.append(i)
                if len(pending) > LAG:
                    phase_c_fin(pending.pop(0))

            # tri work trails the input stream by one chunk: while chunk g
            # streams in (+ colsum matmuls), the PE also runs phase-C of
            # the tiles of chunk g-1.
            tri_cursor = 0
            for g in range(len(CHS)):
                s_ps = [
                    ps_s.tile([CHS[g], HALF], F32, name=f"sps{g}_{h}",
                              tag="s")
                    for h in range(NH)
                ]
                lim = COFF[g]  # phase-C may cover all tiles of prior chunks
                start = tri_cursor
                for j in range(CHS[g]):
                    target = start + (lim - start) * (j + 1) // CHS[g]
                    while tri_cursor < target:
                        emit_tri(tri_cursor)
                        tri_cursor += 1
                    load_and_colsum(g, j, s_ps)
                    if g == 0 and j == 3:
                        # big constant: defer behind the first x tiles so it
                        # doesn't delay the pipeline start; 4-way split so it
                        # lands before the first carry matmul needs it
                        for s in range(4):
                            rs = slice(s * NT // 2, (s + 1) * NT // 2)
                            nc.sync.dma_start(
                                carb2_sb[rs, :], carb2_d.ap()[rs, :]
                            )
                assemble_s2(g, s_ps)
            while tri_cursor < NT:
                emit_tri(tri_cursor)
                tri_cursor += 1
            while pending:
                phase_c_fin(pending.pop(0))

    nc.compile()
    return nc


_NC_CACHE: list = []


def _get_nc() -> bass.Bass:
    if not _NC_CACHE:
        _NC_CACHE.append(_build_nc())
    return _NC_CACHE[0]


def _split_bf16(x: np.ndarray):
    """Re-encode fp32 x as bf16 hi/lo pair (error <= ~2^-18 relative)."""
    xh = x.astype(ml_dtypes.bfloat16)
    xl = (x - xh.astype(np.float32)).astype(ml_dtypes.bfloat16)
    return xh, xl


def _run(x: np.ndarray, **kwargs):
    x = np.ascontiguousarray(np.asarray(x), dtype=np.float32)
    assert x.shape == (B, T, C), x.shape
    nc = _get_nc()
    xh, xl = _split_bf16(x)
    in_maps = [{"xh": xh[b], "xl": xl[b]} for b in range(B)]
    return run_bass_kernel_spmd(nc, in_maps, core_ids=list(range(B)), **kwargs)


def kernel(x: np.ndarray) -> np.ndarray:
    res = _run(x)
    return np.stack([r["y"] for r in res.results], axis=0)


# revision 61
# speedup vs baseline: 1.0247x; 1.0247x over previous
"""CausalBoW (causal mean pooling) Trainium2 Bass kernel.

y[b, t, :] = mean(x[b, 0:t+1, :]) = cumsum(x, axis=1) / (t+1)

Full input x: [8, 4096, 1024] f32. Sharded batch-parallel: one batch of
[4096, 1024] per NeuronCore (8 cores).

The fp32 input is re-encoded (losslessly up to ~2^-18 relative) as a pair
of bf16 tensors xh = bf16(x), xl = bf16(x - xh). This keeps HBM traffic
identical to streaming fp32 x (2 x 8 MiB vs 16 MiB per core) while letting
every matmul run at the PE's full 1 column/cycle bf16 rate (fp32 matmul is
4x slower, fp32r truncates to 11 mantissa bits). All matmul weights are
exactly 0/1 so the products are exact; accumulation is fp32 in PSUM.

Per-core algorithm: blocked scan, T on partitions in 32 tiles of 128 rows,
processed in 4 chunks of 8 tiles so compute/output of chunk g overlaps the
input stream of chunk g+1:
  per chunk g:
    per tile i in chunk: DMA xh/xl tile in;
      PSUM Sg[i%8, :] += colsum(xh_i) + colsum(xl_i)  (one-hot selector MM)
    evict Sg into s2 rows [g*16, g*16+8) as bf16-hi and [g*16+8, g*16+16)
      as bf16-lo  (s2 interleaves chunk-blocks of hi/lo tile sums)
    per tile i in chunk, per 512-half (PSUM accumulation group):
      z  = tri.T @ xh_i + tri.T @ xl_i        local inclusive cumsum
      z += carb2_i.T @ s2[0:16*(g+1)]         carry (hi+lo folded, one MM)
      y_i = z * (1/(t+1))    per-partition scale on PSUM->SBUF evict
      DMA y half out.
"""

import sys

for _p in ("/opt/trn_rl_repo",):
    if _p not in sys.path:
        sys.path.insert(0, _p)

import ml_dtypes
import numpy as np

import concourse.bass as bass
import concourse.mybir as mybir
import concourse.tile as tile
from concourse import bacc
from concourse.bass_utils import run_bass_kernel_spmd

B, T, C = 8, 4096, 1024
P = 128            # partition tile rows
NT = T // P        # 32 row-tiles
HALF = 512         # PSUM bank free-dim for f32
NH = C // HALF     # 2 halves
CHS = [8, 8, 8, 4, 4]          # chunk sizes (tiles); small final chunks
assert sum(CHS) == NT          # shorten the drain after the input stream
COFF = [sum(CHS[:b]) for b in range(len(CHS))]   # chunk tile offsets

F32 = mybir.dt.float32
BF16 = mybir.dt.bfloat16


def _build_nc() -> bass.Bass:
    nc = bacc.Bacc(trn_type="TRN2")

    xh = nc.declare_dram_parameter("xh", [T, C], BF16, isOutput=False)
    xl = nc.declare_dram_parameter("xl", [T, C], BF16, isOutput=False)
    y = nc.declare_dram_parameter("y", [T, C], F32, isOutput=True)

    # Constants baked into the NEFF (all weights exactly 0/1).
    # lhsT for local inclusive cumsum: out = lhsT.T @ rhs, want
    # out[t, c] = sum_{s<=t} x[s, c] => lhsT[s, t] = 1 iff s <= t.
    tri_np = np.triu(np.ones((P, P), dtype=ml_dtypes.bfloat16))
    # carry weights over the interleaved tile-sum table s2:
    # s2 row k holds: chunk c0 = k//16, j8 = k%16; tile j = c0*8 + (j8%8);
    # j8 < 8 -> hi part of S_j, else lo part. carry_i needs sum of both
    # parts for all j < i:
    #   carb2[k, i*128 + m] = 1 iff (k//16)*8 + (k%16)%8 < i.
    jmap = np.empty(NT * 2, dtype=np.int64)
    for b, (off, cb) in enumerate(zip(COFF, CHS)):
        k0 = 2 * off
        jmap[k0 : k0 + 2 * cb] = off + (np.arange(2 * cb) % cb)
    carb2_np = (
        (jmap[:, None, None] < np.arange(NT)[None, :, None])
        * np.ones((1, 1, P))
    ).reshape(NT * 2, NT * P).astype(ml_dtypes.bfloat16)
    # banded one-hot-column selector for routing colsum(x_i) into PSUM row
    # j: bnd8[:, (7-j) : (7-j+cb)] has ones exactly in column j.
    bnd8_np = np.zeros((P, 15), dtype=ml_dtypes.bfloat16)
    bnd8_np[:, 7] = 1.0
    # inv[p, i] = 1 / (i*128 + p + 1)
    inv_np = (
        1.0 / np.arange(1, T + 1, dtype=np.float64)
    ).astype(np.float32).reshape(NT, P).T.copy()

    tri_d = nc.inline_tensor(tri_np, name="tri_c")
    carb2_d = nc.inline_tensor(carb2_np, name="carb2_c")
    bnd8_d = nc.inline_tensor(bnd8_np, name="bnd8_c")
    inv_d = nc.inline_tensor(inv_np, name="inv_c")

    with tile.TileContext(nc) as tc:
        with (
            tc.tile_pool(name="consts", bufs=1) as cpool,
            tc.tile_pool(name="xpool", bufs=2 * NT) as xpool,
            tc.tile_pool(name="ypoolA", bufs=4) as ypoolA,
            tc.tile_pool(name="ypoolB", bufs=4) as ypoolB,
            tc.tile_pool(name="s2p", bufs=1) as s2p,
            tc.tile_pool(name="stmp", bufs=2) as stmp,
            tc.tile_pool(name="ps_s", bufs=4, space="PSUM") as ps_s,
            tc.tile_pool(name="ps_z", bufs=4, space="PSUM") as ps_z,
        ):
            bnd8_sb = cpool.tile([P, 15], BF16)
            nc.sync.dma_start(bnd8_sb[:], bnd8_d.ap())
            tri_sb = cpool.tile([P, P], BF16)
            nc.sync.dma_start(tri_sb[:], tri_d.ap())
            inv_sb = cpool.tile([P, NT], F32)
            nc.sync.dma_start(inv_sb[:], inv_d.ap())
            carb2_sb = cpool.tile([NT * 2, NT * P], BF16)

            s2_sb = s2p.tile([NT * 2, C], BF16)

            xhs, xls = [None] * NT, [None] * NT

            def load_and_colsum(g: int, j: int, s_ps):
                """DMA tile j of chunk g in, accumulate its column sums."""
                i = COFF[g] + j
                cb = CHS[g]
                # Each DMA engine runs at ~1/16 of HBM bandwidth, so a whole
                # 256 KiB tile on one queue has ~11 us latency. Split the
                # first tiles so the pipeline starts promptly.
                nsplit = 4 if i < 1 else 1
                ps = P // nsplit
                xht = xpool.tile([P, C], BF16, name=f"xht{i}", tag="x")
                xlt = xpool.tile([P, C], BF16, name=f"xlt{i}", tag="x")
                for s in range(nsplit):
                    rs = slice(s * ps, (s + 1) * ps)
                    gs = slice(i * P + s * ps, i * P + (s + 1) * ps)
                    # first tile: fan triggers over both HWDGE engines to
                    # dodge the sync-queue trigger serialization at start
                    heng = nc.scalar if nsplit > 1 else nc.sync
                    heng.dma_start(xht[rs, :], xh.ap()[gs, :])
                    nc.sync.dma_start(xlt[rs, :], xl.ap()[gs, :])
                xhs[i], xls[i] = xht, xlt
                lhs_j = bnd8_sb[:, 7 - j : 7 - j + cb]
                for h in range(NH):
                    hs = slice(h * HALF, (h + 1) * HALF)
                    nc.tensor.matmul(
                        s_ps[h][:], lhsT=lhs_j, rhs=xht[:, hs],
                        start=(j == 0), stop=False,
                    )
                    nc.tensor.matmul(
                        s_ps[h][:], lhsT=lhs_j, rhs=xlt[:, hs],
                        start=False, stop=(j == cb - 1),
                    )

            def assemble_s2(g: int, s_ps):
                """Evict chunk-g tile-sums into s2 rows as bf16 hi/lo.

                DVE writes must start at partition 0/32/64/96, so evict to
                base-0 temporaries and DMA (any partition) into s2 rows.
                """
                cb = CHS[g]
                r0 = 2 * COFF[g]
                th = stmp.tile([cb, C], BF16, name=f"th{g}", tag="th")
                tl = stmp.tile([cb, C], BF16, name=f"tl{g}", tag="tl")
                for h in range(NH):
                    hs = slice(h * HALF, (h + 1) * HALF)
                    nc.scalar.copy(th[:, hs], s_ps[h][:])
                    nc.vector.tensor_sub(tl[:, hs], s_ps[h][:], th[:, hs])
                # scalar HWDGE: the th copy runs on ACT, so its hop trigger
                # follows in the same queue with no cross-engine sem, and
                # ACT's trigger queue is short (sync carries the input
                # stream)
                nc.scalar.dma_start(s2_sb[r0 : r0 + cb, :], th[:])
                nc.scalar.dma_start(s2_sb[r0 + cb : r0 + 2 * cb, :], tl[:])

            zps = [None] * NT

            def phase_c_tri(i: int):
                """Local-cumsum matmuls for tile i (no carry dependency)."""
                zps[i] = []
                for h in range(NH):
                    zp = ps_z.tile([P, HALF], F32, name=f"zp{i}_{h}", tag="z")
                    zps[i].append(zp)
                    hs = slice(h * HALF, (h + 1) * HALF)
                    nc.tensor.matmul(
                        zp[:], lhsT=tri_sb[:], rhs=xhs[i][:, hs],
                        start=True, stop=False,
                    )
                    nc.tensor.matmul(
                        zp[:], lhsT=tri_sb[:], rhs=xls[i][:, hs],
                        start=False, stop=(i == 0),
                    )

            def phase_c_fin(i: int):
                """Carry matmul + scale-evict + store for tile i."""
                # carry table prefix: rows for all tiles < i. The first tile
                # of a chunk needs nothing from its own chunk's block.
                b = max(bb for bb in range(len(CHS)) if COFF[bb] <= i)
                k2 = 2 * COFF[b] + (2 * CHS[b] if i > COFF[b] else 0)
                for h in range(NH):
                    zp = zps[i][h]
                    hs = slice(h * HALF, (h + 1) * HALF)
                    if i > 0:
                        nc.tensor.matmul(
                            zp[:],
                            lhsT=carb2_sb[0:k2, i * P : (i + 1) * P],
                            rhs=s2_sb[0:k2, hs],
                            start=False, stop=True,
                        )
                    # evict with per-partition 1/(t+1) scale; split halves
                    # across ACT and DVE. The h0 store is triggered from the
                    # scalar engine itself (HWDGE) to offload the sync queue.
                    if h == 0:
                        yt = ypoolA.tile([P, HALF], F32, name=f"yta{i}",
                                         tag="ya")
                        nc.scalar.mul(yt[:], zp[:], inv_sb[:, i : i + 1])
                        dma_eng = nc.gpsimd
                    else:
                        yt = ypoolB.tile([P, HALF], F32, name=f"ytb{i}",
                                         tag="yb")
                        nc.vector.tensor_scalar_mul(
                            yt[:], zp[:], inv_sb[:, i : i + 1]
                        )
                        dma_eng = nc.gpsimd
                    # tail: the input stream is done, sync is idle — use it
                    # for the last stores so they don't queue on scalar
                    if i >= NT - 2:
                        dma_eng = nc.sync
                    # split the last tiles' stores to shorten the tail
                    nsplit = 2 if i >= NT - 2 else 1
                    ps = P // nsplit
                    for s in range(nsplit):
                        rs = slice(s * ps, (s + 1) * ps)
                        gs = slice(i * P + s * ps, i * P + (s + 1) * ps)
                        dma_eng.dma_start(y.ap()[gs, hs], yt[rs, :])

            # Software pipeline: interleave chunk g's input stream + column
            # sums with chunk g-1's compute at tile granularity, so the
            # in-order PE queue always has dense work between DMA-paced
            # column-sum matmuls and the s2 assembly latency is hidden.
            # phase-C is emitted with a two-tile lag between the carry-free
            # tri matmuls and the carry+evict part, so the PE queue always
            # has independent work while the s2 carry table assembles.
            LAG = 1
            pending: list = []

            def emit_tri(i: int):
                phase_c_tri(i)
                pending.append(i)
                if len(pending) > LAG:
                    phase_c_fin(pending.pop(0))

            # tri work trails the input stream by one chunk: while chunk g
            # streams in (+ colsum matmuls), the PE also runs phase-C of
            # the tiles of chunk g-1.
            tri_cursor = 0
            for g in range(len(CHS)):
                s_ps = [
                    ps_s.tile([CHS[g], HALF], F32, name=f"sps{g}_{h}",
                              tag="s")
                    for h in range(NH)
                ]
                lim = COFF[g]  # phase-C may cover all tiles of prior chunks
                start = tri_cursor
                for j in range(CHS[g]):
                    target = start + (lim - start) * (j + 1) // CHS[g]
                    while tri_cursor < target:
                        emit_tri(tri_cursor)
                        tri_cursor += 1
                    load_and_colsum(g, j, s_ps)
                    if g == 0 and j == 3:
                        # big constant: defer behind the first x tiles so it
                        # doesn't delay the pipeline start; 4-way split so it
                        # lands before the first carry matmul needs it
                        for s in range(4):
                            rs = slice(s * NT // 2, (s + 1) * NT // 2)
                            nc.sync.dma_start(
                                carb2_sb[rs, :], carb2_d.ap()[rs, :]
                            )
                assemble_s2(g, s_ps)
            while tri_cursor < NT:
                emit_tri(tri_cursor)
                tri_cursor += 1
            while pending:
                phase_c_fin(pending.pop(0))

    nc.compile()
    return nc


_NC_CACHE: list = []


def _get_nc() -> bass.Bass:
    if not _NC_CACHE:
        _NC_CACHE.append(_build_nc())
    return _NC_CACHE[0]


def _split_bf16(x: np.ndarray):
    """Re-encode fp32 x as bf16 hi/lo pair (error <= ~2^-18 relative)."""
    xh = x.astype(ml_dtypes.bfloat16)
    xl = (x - xh.astype(np.float32)).astype(ml_dtypes.bfloat16)
    return xh, xl


def _run(x: np.ndarray, **kwargs):
    x = np.ascontiguousarray(np.asarray(x), dtype=np.float32)
    assert x.shape == (B, T, C), x.shape
    nc = _get_nc()
    xh, xl = _split_bf16(x)
    in_maps = [{"xh": xh[b], "xl": xl[b]} for b in range(B)]
    return run_bass_kernel_spmd(nc, in_maps, core_ids=list(range(B)), **kwargs)


def kernel(x: np.ndarray) -> np.ndarray:
    res = _run(x)
    return np.stack([r["y"] for r in res.results], axis=0)
